# revision 12
# baseline (speedup 1.0000x reference)
"""Trainium2 Bass kernel for DistillLossSimpleMSE (segment_reduce).

Math (per object o, with uniform segments of P points):
    x   = net_out[o*P:(o+1)*P]                [P, D]
    m   = mask_pts[o]                         [M, P] in {0,1}
    e   = nan_to_num(mask_embs[o*M:(o+1)*M])  [M, D]
    sum_sq = sum_m [ sum_p m*||x_p||^2 + cnt_m*||e_m||^2 - 2 e_m . (sum_p m x_p) ]
    out = sum_sq / (D * total_points)

Sharding: object-parallel, 1 object per core (8 objects, 8 cores).

Device kernel per core accumulates in PSUM over all P points:
    acc[32, 257] = m^T.T @ [x | x*x | 1]      (f32r matmuls, 1 cyc/col)
      cols   0:128 -> mx[m, d],  cols 128:256 -> sum_p m x^2,  col 256 -> cnt
Host does the tiny per-mask finale with the embeddings.

Mask is DMA'd in its contiguous [128, 16384] flat view (full-partition DMA
bandwidth) and transposed on-chip through the PE; one [128,128] transpose
yields the lhsT mask columns for 4 point-chunks (strided AP picks quarter q).

TRN2 PE matmuls can carry at most ONE semaphore wait, so the kernel is
structured to give every PE instruction at most one fresh cross-engine
dependency: persistent rhs buffers with the `ones` column written once at
startup, transposes packed 4-per-PSUM-bank so mask copies land on ACT with
ticks the PE has already observed, and tiny "sink" transposes that absorb
the x-DMA and square-op semaphore ticks before the real matmuls run.
"""

import os

import numpy as np

import bass_rust
import concourse.bass as bass
import concourse.mybir as mybir
import concourse.tile as tile
from concourse.bass_utils import run_bass_kernel_spmd

N_CORES = 8
N_OBJ, P, M, D = 8, 65536, 32, 128

VIEW_P = 128                 # mask flat view partitions
VIEW_F = M * P // VIEW_P     # 16384 view cols; view[r, f] = mask[r//4, (r%4)*16384 + f]
BLK = 1024                   # view cols per super-block
NBLK = VIEW_F // BLK         # 16
CPR = BLK // 128             # 8 chunks (of 128 points) per (block, quarter)
OUTC = 2 * D + 2             # 258 output cols: [mx | m@x^2 | cnt | pad] (f32r needs even free dims)
NRHS = 6                     # persistent rhs buffers

F32 = mybir.dt.float32
F32R = mybir.dt.float32r
I32 = mybir.dt.int32

LAST = None      # BassKernelResults of the most recent run (for test harness)
_NC_CACHE = {}


def _build_nc():
    nc = bass.Bass()
    # x declared float32r (same bit layout as f32) so DMA feeds the f32r
    # matmuls directly without a rounding pass.
    x = nc.dram_tensor("x", [P, D], F32R, kind="ExternalInput")
    mask = nc.dram_tensor("mask", [VIEW_P, VIEW_F], I32, kind="ExternalInput")
    out = nc.dram_tensor("out", [M, OUTC], F32, kind="ExternalOutput")
    # Keeps the sink transposes alive (and observable) in case of DCE.
    sink = nc.dram_tensor("sink", [1, 1], F32, kind="ExternalOutput")

    # x chunk view: [512 chunks, 128 points, 128 d]
    xv = x[:, :].rearrange("(a p) d -> a p d", p=128)

    with tile.TileContext(nc) as tc:
        with (
            tc.tile_pool(name="singles", bufs=1) as singles,
            tc.tile_pool(name="psingles", bufs=1, space="PSUM") as psingles,
        ):
            # All tiles are persistent (allocated once, manually rotated).
            # Pool-reallocated tiles go through Tile's release machinery,
            # which emits same-engine release waits; on PE matmuls (1-wait
            # limit) those collide with the real cross-engine wait and break
            # codegen. Persistent tiles get same-engine deps elided.
            # Constants come from inline DRAM tensors via DMA: memset/affine
            # -select cannot produce float32r (ISA + verifier restrictions).
            ident_const = nc.inline_tensor(np.eye(128, dtype=np.float32), name="identc")
            ident = singles.tile([128, 128], F32R, tag="ident")
            nc.sync.dma_start(out=ident, in_=ident_const[:, :].bitcast(F32R))

            ones_const = nc.inline_tensor(
                np.ones((128, CPR, 2), dtype=np.float32), name="onesc"
            )

            # rhs buffers; ones/pad columns written exactly once so the
            # in-loop matmuls never depend on their producer.
            rhs_bufs = []
            for j in range(NRHS):
                rb = singles.tile([128, CPR, OUTC], F32R, name=f"rhsbuf{j}", tag=f"rhsbuf{j}")
                nc.sync.dma_start(
                    out=rb[:, :, 2 * D:2 * D + 2],
                    in_=ones_const[:, :, :].bitcast(F32R),
                )
                rhs_bufs.append(rb)

            mi_bufs = [singles.tile([VIEW_P, BLK], I32, name=f"mi{j}", tag=f"mi{j}") for j in range(2)]
            mf_bufs = [singles.tile([VIEW_P, BLK], F32R, name=f"mf{j}", tag=f"mf{j}") for j in range(2)]
            mt_bufs = [singles.tile([VIEW_P, BLK], F32R, name=f"mt{j}", tag=f"mt{j}") for j in range(2)]
            ps4_bufs = [
                psingles.tile([128, 4, 128], F32R, name=f"ps4{j}", tag=f"ps4{j}") for j in range(2)
            ]

            acc = psingles.tile([M, OUTC], F32, tag="acc")
            scr = psingles.tile([128, 128], F32R, tag="scratch")

            n_mm = NBLK * 4 * CPR
            k = 0
            jbuf = 0
            for i in range(NBLK):
                mi = mi_bufs[i % 2]
                nc.sync.dma_start(out=mi, in_=mask[:, i * BLK:(i + 1) * BLK])
                mf = mf_bufs[i % 2]
                nc.gpsimd.tensor_copy(mf, mi)

                # Transpose mask block through PE, 4 tiles per PSUM bank, then
                # one ACT copy per bank: mt[:, t*128+r] = mask^T over points,
                # where view col r = m*4 + q.
                mt = mt_bufs[i % 2]
                for h in range(2):
                    ps4 = ps4_bufs[h]
                    for tt in range(4):
                        t = h * 4 + tt
                        nc.tensor.transpose(
                            ps4[:, tt, :],
                            mf[:, t * 128:(t + 1) * 128],
                            ident,
                        )
                    nc.scalar.copy(
                        mt[:, h * 512:(h + 1) * 512],
                        ps4.rearrange("p t d -> p (t d)"),
                    )
                mtv = mt.rearrange("p (t m q) -> p t q m", t=CPR, m=M, q=4)

                for q in range(4):
                    rhs = rhs_bufs[jbuf]
                    jbuf = (jbuf + 1) % NRHS
                    a0 = q * (P // 128 // 4) + i * CPR
                    nc.sync.dma_start(
                        out=rhs[:, :, 0:D],
                        in_=xv[a0:a0 + CPR, :, :].rearrange("c p d -> p c d"),
                    )
                    sq_out = rhs[:, :, D:2 * D]
                    x_in = rhs[:, :, 0:D]
                    if (i + q) % 2 == 0:
                        nc.vector.tensor_mul(sq_out, x_in, x_in)
                    else:
                        nc.scalar.activation(
                            sq_out, x_in, mybir.ActivationFunctionType.Square
                        )
                    # Sink transposes: absorb the x-DMA tick and the square
                    # tick so the real matmuls only wait on the ACT mask copy.
                    nc.tensor.transpose(
                        scr[0:32, 0:32], rhs[:, 0, 0:32], ident[:, 0:32]
                    )
                    nc.tensor.transpose(
                        scr[0:32, 0:32], rhs[:, 0, D:D + 32], ident[:, 0:32]
                    )
                    for c in range(CPR):
                        nc.tensor.matmul(
                            acc[:, :],
                            lhsT=mtv[:, c, q, :],
                            rhs=rhs[:, c, :],
                            start=(k == 0),
                            stop=(k == n_mm - 1),
                        )
                        k += 1

            outs = singles.tile([M, OUTC], F32, tag="outs")
            nc.vector.tensor_copy(outs, acc)
            nc.sync.dma_start(out=out[:, :], in_=outs)
            sinks = singles.tile([1, 1], F32, tag="sinks")
            nc.vector.tensor_copy(sinks, scr[0:1, 0:1])
            nc.sync.dma_start(out=sink[:, :], in_=sinks)
    # Split multi-wait instructions into EventSemaphore + instruction to
    # satisfy the TRN2 1-wait-per-instruction codegen limit.
    bass_rust.generate_event_semaphores(nc)
    return nc


def _get_nc():
    if "nc" not in _NC_CACHE:
        _NC_CACHE["nc"] = _build_nc()
    return _NC_CACHE["nc"]


def kernel(net_out, pt_offset, mask_embs, mask_pts, logit_scale):
    global LAST
    net_out = np.ascontiguousarray(np.asarray(net_out, dtype=np.float32))
    mask_pts = np.ascontiguousarray(np.asarray(mask_pts, dtype=np.int32))
    mask_embs = np.asarray(mask_embs, dtype=np.float32)

    nc = _get_nc()
    in_maps = [
        {
            "x": net_out[o * P:(o + 1) * P],
            "mask": mask_pts[o].reshape(VIEW_P, VIEW_F),
        }
        for o in range(N_CORES)
    ]
    trace = os.environ.get("KBENCH_TRACE", "0") == "1"
    res = run_bass_kernel_spmd(nc, in_maps, list(range(N_CORES)), trace=trace)
    LAST = res

    accs = np.stack([np.asarray(res.results[o]["out"]) for o in range(N_CORES)])
    mx = accs[:, :, 0:D].astype(np.float64)        # [8, 32, 128]
    sx2 = accs[:, :, D:2 * D].astype(np.float64)   # [8, 32, 128]
    cnt = accs[:, :, 2 * D].astype(np.float64)     # [8, 32]

    emb = np.nan_to_num(
        mask_embs.reshape(N_OBJ, M, D).astype(np.float64),
        nan=0.0, posinf=0.0, neginf=0.0,
    )
    t1 = sx2.sum(-1)
    t2 = cnt * (emb * emb).sum(-1)
    t3 = 2.0 * (emb * mx).sum(-1)
    sum_sq = (t1 + t2 - t3).sum()
    total = cnt.sum()
    val = sum_sq / (D * total) if total > 0 else 0.0
    return np.float32(val)
